# revision 6
# baseline (speedup 1.0000x reference)
"""Continual-attention Trainium2 kernel v3 (8 NeuronCores, SPMD).

Sharding: core c -> batch b = c//2, head-group g = c%2 (4 heads each).

Per (b,h): S^T[k,q] = K Q^T on PE with 64 data contraction rows (scaled by
sqrt(A), A = 1024*log2(e)/8) plus 64 mask rows folded into the same matmul:
Q rows 64-127 hold per-chunk indicators for test queries, K rows 64-127 hold
-60000*[k > attach[b,n]] steps, so masked logits go hugely negative in PSUM.
The exp+PSUM-drain pass is split between ScalarE (true exp via activation,
scale=ln2/1024) and DVE (Schraudolph: round(A*S + B) as uint16 = the fp16 bit
pattern of ~exp(logit); negatives saturate to 0 killing masked entries).
Remaining diagonal/chunk triangle masks are 0/1 multiplies on DVE.
O^T[65,q] (64 dims + denominator row via ones column in V) accumulates on PE
into [65,1024] PSUM tiles; normalization + final transpose happen on host.
"""

import sys

sys.path.insert(0, "/opt/trn_rl_repo")

import numpy as np

B, L, H, D = 4, 2048, 8, 64
TRAIN = 1536
TEST = L - TRAIN            # 512
NCH = 64                    # test chunks
CH = TEST // NCH            # 8
HPC = 4                     # heads per core
NCORES = 8
KT = L // 128               # 16 k-tiles

VW = KT * 65                # v columns per head
SPW = 512                   # PSUM S-tile width (1 bank)
PIPE = 6                    # sp tiles held back before AV

A_EXP = 1024.0 * np.log2(np.e) * 0.125     # PSUM = A_EXP * S
SQA = float(np.sqrt(A_EXP))                # folded into both Q and K
SC_SCALE = float(np.log(2.0) / 1024.0)     # ScalarE: exp(PSUM*SC_SCALE)
DVE_B = 15360.0 - 58.7                     # log-mean-centered Schraudolph bias
MASKVAL = -60000.0

LAST_RESULT = None
_PROG = None


def _split_multi_waits(nc, mybir):
    """This container's walrus accepts at most one semaphore wait per
    instruction; Tile's tail drains can carry several. Hoist extras onto
    NoOps inserted immediately before, on the same engine."""
    for f in nc.m.functions:
        for bb in f.blocks:
            insts = list(bb.instructions)
            out = []
            changed = False
            for inst in insts:
                si = inst.sync_info
                if si is not None and len(si.on_wait) > 1:
                    waits = list(si.on_wait)
                    for w in waits[:-1]:
                        nop = mybir.InstNoOp(
                            name=f"waitnop-{nc.next_id()}", ins=[], outs=[]
                        )
                        nop.engine = inst.engine
                        nop.sync_info = mybir.SyncInfo(on_wait=[w], on_update=[])
                        out.append(nop)
                    inst.sync_info = mybir.SyncInfo(
                        on_wait=[waits[-1]], on_update=list(si.on_update)
                    )
                    changed = True
                out.append(inst)
            if changed:
                bb.instructions = out


def _pieces(h_unused=None):
    """Per-head piece list in processing order.
    Returns [(kp, qs, w, mask, start, stop, half), ...] where qs is the
    global q start, mask in (None,'diag','chunk'), start/stop are the AV
    accumulation flags, half = qs // 1024 region of the av tile pair."""
    out = []
    for gq in range(4):
        if gq < 3:
            kps = list(range(4 * (gq + 1)))
        else:
            kps = list(range(16))
        for i, kp in enumerate(kps):
            if kp >= 12:
                if kp % 2 == 1:
                    continue  # odd test tiles are folded into the even pair
                j = (kp - 12) // 2
                out.append(
                    dict(kp=kp, qs=128 * kp, w=256, mask="chunk", gq=gq,
                         lhs=2048 + 128 * j, rhs=2048 + 256 * j,
                         start=False, stop=(kp == 14))
                )
                continue
            if False:
                pass
            else:
                off = max(0, 128 * kp - 512 * gq)
                qs = 512 * gq + off
                w = 512 - off
                mask = "diag" if kp // 4 == gq else None
            out.append(
                dict(kp=kp, qs=qs, w=w, mask=mask, gq=gq,
                     start=(i == 0), stop=(i == len(kps) - 1))
            )
    return out


def _pack(pieces):
    """Pack pieces into [128, SPW] sp tiles; no piece crosses a 512-col
    PSUM bank. Returns list of tiles, each a list of (piece, pos)."""
    tiles = []
    cur = []
    pos = 0
    for p in pieces:
        w = p["w"]
        bank_rem = -pos % 512
        if 0 < bank_rem < w:
            pos += bank_rem
        if pos + w > SPW:
            tiles.append(cur)
            cur = []
            pos = 0
        cur.append((p, pos))
        pos += w
    if cur:
        tiles.append(cur)
    return tiles


def _build_program():
    import concourse.bass as bass
    import concourse.mybir as mybir
    import concourse.tile as tile

    f32 = mybir.dt.float32
    fp16 = mybir.dt.float16
    u16 = mybir.dt.uint16
    Exp = mybir.ActivationFunctionType.Exp

    nc = bass.Bass()

    qx_d = nc.dram_tensor("qx", [HPC, 128, L + 512], fp16, kind="ExternalInput")
    kx_d = nc.dram_tensor("kx", [HPC, 128, L + 256], fp16, kind="ExternalInput")
    vw_d = nc.dram_tensor("vw", [HPC, 128, VW], fp16, kind="ExternalInput")
    msk_d = nc.dram_tensor("msk", [128, 256], fp16, kind="ExternalInput")
    ot_d = nc.dram_tensor("ot", [HPC, 65, L], fp16, kind="ExternalOutput")

    with tile.TileContext(nc) as tc:
        with (
            tc.tile_pool(name="consts", bufs=1) as consts,
            tc.tile_pool(name="heads", bufs=4) as heads,
            tc.tile_pool(name="ptp", bufs=8) as ptp,
            tc.tile_pool(name="osbp", bufs=5) as osbp,
            tc.tile_pool(name="spp", bufs=6, space="PSUM") as spp,
            tc.tile_pool(name="avp", bufs=2, space="PSUM") as avp,
        ):
            # ---- PE clock warm-up + ACT table preload ---------------------
            warm_sb = consts.tile([128, 128], fp16, name="warm_sb")
            nc.gpsimd.memset(warm_sb, 0.0)
            warm_ps = spp.tile([128, SPW], f32, tag="sp", name="warm_ps")
            warm_pt = consts.tile([128, 128], fp16, name="warm_pt")
            nc.scalar.activation(warm_pt, warm_sb, Exp, scale=SC_SCALE)
            for _ in range(38):
                nc.tensor.matmul(
                    warm_ps[:, 0:128], lhsT=warm_sb, rhs=warm_sb,
                    start=True, stop=True, skip_group_check=True,
                )

            # ---- input DMAs ----------------------------------------------
            msk_sb = consts.tile([128, 256], fp16)
            qx_sbs, kx_sbs, vw_sbs = [], [], []
            for h in range(HPC):
                qx_sbs.append(heads.tile([128, L + 512], fp16, tag="qx", name=f"qx{h}"))
                kx_sbs.append(heads.tile([128, L + 256], fp16, tag="kx", name=f"kx{h}"))
                vw_sbs.append(heads.tile([128, VW], fp16, tag="vw", name=f"vw{h}"))

            nc.gpsimd.dma_start(out=kx_sbs[0][:, 0:512], in_=kx_d.ap()[0][:, 0:512])
            nc.gpsimd.dma_start(out=qx_sbs[0][:, 0:1024], in_=qx_d.ap()[0][:, 0:1024])
            nc.scalar.dma_start(out=msk_sb, in_=msk_d.ap())
            nc.gpsimd.dma_start(out=kx_sbs[0][:, 512:2304], in_=kx_d.ap()[0][:, 512:2304])
            nc.gpsimd.dma_start(out=qx_sbs[0][:, 1024:2560], in_=qx_d.ap()[0][:, 1024:2560])
            nc.gpsimd.dma_start(out=vw_sbs[0], in_=vw_d.ap()[0])
            for h in range(1, HPC):
                nc.gpsimd.dma_start(out=kx_sbs[h], in_=kx_d.ap()[h])
                nc.gpsimd.dma_start(out=qx_sbs[h], in_=qx_d.ap()[h])
                nc.gpsimd.dma_start(out=vw_sbs[h], in_=vw_d.ap()[h])

            mdiag = msk_sb[:, 0:128]
            mchunk = msk_sb[:, 128:256]

            # greedy engine balancing for converts / copies
            eng_t = {"sc": 0.0, "ve": 0.0}

            def conv_cost(eng, cols):
                if eng == "sc":
                    return (cols + 352) / 1.2
                return cols / 0.96 + 150.0

            pending = []
            osb_i = 0

            def convert(pt, sp, lo, hi, force_ve=False):
                """Emit the exp/PSUM-drain for sp[:, lo:hi] on the engine with
                the lower projected load. Halves containing masked pieces are
                pinned to DVE so the subsequent mask mul is ordered by the
                engine queue rather than a cross-engine semaphore."""
                cols = hi - lo
                if cols <= 0:
                    return
                if not force_ve and \
                   eng_t["sc"] + conv_cost("sc", cols) <= \
                   eng_t["ve"] + conv_cost("ve", cols):
                    eng_t["sc"] += conv_cost("sc", cols)
                    nc.scalar.activation(
                        pt.bitcast(mybir.dt.float16)[:, lo:hi],
                        sp[:, lo:hi], Exp, scale=SC_SCALE,
                    )
                else:
                    eng_t["ve"] += conv_cost("ve", cols)
                    nc.vector.tensor_scalar_add(pt[:, lo:hi], sp[:, lo:hi], DVE_B)

            def emit_avs(rec):
                nonlocal osb_i
                pt_, av_, h_, tlist = rec
                vw_sb = vw_sbs[h_]
                ptf = pt_.bitcast(mybir.dt.float16)
                for p, pos in tlist:
                    kp, qs, w = p["kp"], p["qs"], p["w"]
                    gq = p["gq"]
                    av = av_[gq]
                    nav = 2 if p["mask"] == "chunk" and w == 256 else 1
                    for t in range(nav):
                        nc.tensor.matmul(
                            av[:, qs + 128 * t - 512 * gq:
                               qs + 128 * t - 512 * gq + w // nav],
                            lhsT=vw_sb[:, 65 * (kp + t): 65 * (kp + t) + 65],
                            rhs=ptf[:, pos + 128 * t: pos + 128 * t + w // nav],
                            start=p["start"],
                            stop=p["stop"] and t == nav - 1,
                            skip_group_check=True,
                        )
                    if p["stop"]:
                        osb = osbp.tile([65, 512], mybir.dt.float16,
                                        name=f"osb{osb_i}")
                        osb_i += 1
                        if eng_t["sc"] + conv_cost("sc", 512) <= \
                           eng_t["ve"] + conv_cost("ve", 512):
                            eng_t["sc"] += conv_cost("sc", 512)
                            nc.scalar.copy(osb, av[:, :])
                        else:
                            eng_t["ve"] += conv_cost("ve", 512)
                            nc.vector.tensor_copy(osb, av[:, :])
                        nc.gpsimd.dma_start(
                            out=ot_d.ap()[h_][:, 512 * gq: 512 * (gq + 1)],
                            in_=osb,
                        )

            for h in range(HPC):
                qx, kx = qx_sbs[h], kx_sbs[h]
                avs = {}
                for gq in range(4):
                    avs[gq] = avp.tile([65, 512], f32, tag="av",
                                       name=f"av{h}_{gq}")
                for tlist in _pack(_pieces()):
                    sp = spp.tile([128, SPW], f32, tag="sp")
                    used = tlist[-1][1] + tlist[-1][0]["w"]
                    pt = ptp.tile([128, SPW], u16, tag="pt")
                    for p, pos in tlist:
                        kp, qs, w = p["kp"], p["qs"], p["w"]
                        lo = p.get("lhs", 128 * kp)
                        ro = p.get("rhs", qs)
                        nc.tensor.matmul(
                            sp[:, pos: pos + w],
                            lhsT=kx[:, lo: lo + 128],
                            rhs=qx[:, ro: ro + w],
                            start=True, stop=True, skip_group_check=True,
                        )
                    has_mask = any(p["mask"] for p, pos in tlist)
                    convert(pt, sp, 0, used, force_ve=has_mask)
                    ptf = pt.bitcast(mybir.dt.float16)
                    for p, pos in tlist:
                        if p["mask"] == "diag":
                            nc.vector.tensor_mul(
                                ptf[:, pos: pos + 128],
                                ptf[:, pos: pos + 128], mdiag,
                            )
                        elif p["mask"] == "chunk":
                            for mo in range(0, p["w"], 128):
                                nc.vector.tensor_mul(
                                    ptf[:, pos + mo: pos + mo + 128],
                                    ptf[:, pos + mo: pos + mo + 128], mchunk,
                                )
                    pending.append((pt, avs, h, tlist))
                    while len(pending) > PIPE:
                        emit_avs(pending.pop(0))
            while pending:
                emit_avs(pending.pop(0))

    import concourse.mybir as mybir_mod

    _split_multi_waits(nc, mybir_mod)
    return nc


def _host_inputs(queries, keys, values, attach):
    f16 = np.float16
    p = np.arange(128)
    f = np.arange(128)
    mdiag = (f[None, :] >= p[:, None]).astype(np.float32)
    # test chunk mask: same CH-chunk and causal within the 128-block
    mchunk = ((p[:, None] // CH == f[None, :] // CH) &
              (p[:, None] <= f[None, :])).astype(np.float32)
    msk = np.concatenate([mdiag, mchunk], axis=1)  # [128, 256]

    kg = np.arange(TRAIN)
    qn = np.arange(NCH)
    in_maps = []
    for c in range(NCORES):
        b, g = divmod(c, 2)
        hs = slice(HPC * g, HPC * (g + 1))
        q = queries[b][:, hs, :]          # [L, 4, D]
        k = keys[b][:, hs, :]
        v = values[b][:, hs, :]
        qt = np.ascontiguousarray(q.transpose(1, 2, 0)) * SQA  # [4, 64, L]
        kt = np.ascontiguousarray(k.transpose(1, 2, 0)) * SQA
        vw = np.empty((HPC, L, 65), np.float32)
        vw[:, :, :64] = v.transpose(1, 0, 2)
        vw[:, :, 64] = 1.0
        vw = np.ascontiguousarray(
            vw.reshape(HPC, KT, 128, 65).transpose(0, 2, 1, 3)
            .reshape(HPC, 128, KT * 65)
        )
        # mask rows: K side = MASKVAL*[k > att_n] on train keys,
        # Q side = [chunk(q) == n] on test queries
        att = attach[b]                                   # [64]
        krow = np.where(kg[None, :] > att[:, None], MASKVAL, 0.0)  # [64,1536]
        qrow = np.zeros((NCH, L), np.float32)
        tq = np.arange(TEST)
        qrow[:, TRAIN:] = (tq[None, :] // CH == qn[:, None]).astype(np.float32)

        qx = np.zeros((HPC, 128, L + 512), np.float32)
        kx = np.zeros((HPC, 128, L + 256), np.float32)
        qx[:, :64, :L] = qt
        qx[:, 64:, :L] = qrow[None]
        kx[:, :64, :L] = kt
        kx[:, 64:, :TRAIN] = krow[None]
        # paired test regions: qx[:, :, 2048+256j] holds Q-test block pair j
        # with even blocks in rows 0-63 and odd blocks in rows 64-127;
        # kx[:, :, 2048+128j] holds K-test tiles 12+2j (rows 0-63) stacked
        # over 12+2j+1 (rows 64-127)
        for j in range(2):
            qblk = qt[:, :, TRAIN + 256 * j: TRAIN + 256 * (j + 1)]  # [4,64,256]
            qx[:, :64, L + 256 * j: L + 256 * j + 128] = qblk[:, :, 0:128]
            qx[:, 64:, L + 256 * j + 128: L + 256 * (j + 1)] = qblk[:, :, 128:256]
            kx[:, :64, L + 128 * j: L + 128 * (j + 1)] = \
                kt[:, :, TRAIN + 256 * j: TRAIN + 256 * j + 128]
            kx[:, 64:, L + 128 * j: L + 128 * (j + 1)] = \
                kt[:, :, TRAIN + 256 * j + 128: TRAIN + 256 * (j + 1)]
        in_maps.append(
            {
                "qx": qx.astype(f16),
                "kx": kx.astype(f16),
                "vw": vw.astype(f16),
                "msk": msk.astype(f16),
            }
        )
    return in_maps


def kernel(queries, keys, values, attach_test_after, train_len):
    global LAST_RESULT, _PROG
    import os

    queries = np.asarray(queries, dtype=np.float32)
    keys = np.asarray(keys, dtype=np.float32)
    values = np.asarray(values, dtype=np.float32)
    attach = np.asarray(attach_test_after).astype(np.int64)
    tl = int(np.asarray(train_len))
    assert queries.shape == (B, L, H, D), queries.shape
    assert tl == TRAIN and attach.shape == (B, NCH)

    from concourse.bass_utils import run_bass_kernel_spmd

    if _PROG is None:
        _PROG = _build_program()

    in_maps = _host_inputs(queries, keys, values, attach)
    trace = bool(int(os.environ.get("KERNEL_TRACE", "0")))
    res = run_bass_kernel_spmd(
        _PROG, in_maps, core_ids=list(range(NCORES)), trace=trace
    )
    LAST_RESULT = res

    out = np.empty((B, L, H * D), np.float32)
    for c in range(NCORES):
        b, g = divmod(c, 2)
        ot = res.results[c]["ot"].astype(np.float32)  # [4, 65, L]
        o = ot[:, :64, :] / ot[:, 64:65, :]           # [4, 64, L]
        out[b, :, 256 * g: 256 * (g + 1)] = (
            o.transpose(2, 0, 1).reshape(L, HPC * D)
        )
    return out
